# revision 5
# baseline (speedup 1.0000x reference)
import sys

sys.path.insert(0, "/opt/trn_rl_repo")

import numpy as np

from concourse import bass, bacc, mybir
import concourse.tile as tile
from concourse.bass_utils import run_bass_kernel_spmd

# Problem constants (hardcoded; kernel.py must be self-contained).
N = 200000
C_IN = 64
C_OUT = 16
K3 = 27
KW = K3 * C_OUT  # 432
OUT_SP = (998, 998, 38)
SENT = OUT_SP[0] * OUT_SP[1] * OUT_SP[2]  # 37848152
NK = N * K3

N_CORES = 8
PER = N // N_CORES            # 25000 points per core
PAD = 25088                   # 196 * 128
NCHUNK = PAD // 128           # 196
G = 7                         # chunks per output group
NGRP = NCHUNK // G            # 28
N_IN_TILES = 4
TILE_F = PAD // N_IN_TILES    # 6272 points per input tile
CHUNKS_PER_TILE = TILE_F // 128  # 49
GRPS_PER_TILE = CHUNKS_PER_TILE // G  # 7

TRACE = False
LAST_RESULTS = None

_NC = None


def _build_nc():
    nc = bacc.Bacc("TRN2", target_bir_lowering=False, debug=False)
    featsT = nc.dram_tensor("featsT", [C_IN, PAD], mybir.dt.float32, kind="ExternalInput")
    w_all = nc.dram_tensor("w_all", [C_IN, KW], mybir.dt.float32, kind="ExternalInput")
    # y[g, p, j*432+o] = contribution row n = (g*7+j)*128 + p
    y = nc.dram_tensor("y", [NGRP, 128, G * KW], mybir.dt.float32, kind="ExternalOutput")

    with tile.TileContext(nc) as tc:
        with (
            tc.tile_pool(name="const", bufs=1) as constp,
            tc.tile_pool(name="inp", bufs=2) as inp,
            tc.tile_pool(name="outp", bufs=3) as outp,
            tc.tile_pool(name="psum", bufs=8, space="PSUM") as psump,
        ):
            w_sb = constp.tile([C_IN, KW], mybir.dt.float32)
            nc.sync.dma_start(w_sb[:], w_all[:])

            copy_engines = ["v", "s"]
            dma_engines = [nc.sync, nc.scalar]

            for t in range(N_IN_TILES):
                ftile = inp.tile([C_IN, TILE_F], mybir.dt.float32)
                nc.sync.dma_start(ftile[:], featsT[:, t * TILE_F:(t + 1) * TILE_F])
                for gg in range(GRPS_PER_TILE):
                    g = t * GRPS_PER_TILE + gg
                    out_sb = outp.tile([128, G * KW], mybir.dt.float32)
                    for j in range(G):
                        c_local = gg * G + j
                        ps = psump.tile([128, KW], mybir.dt.float32)
                        nc.tensor.matmul(
                            ps[:],
                            lhsT=ftile[:, c_local * 128:(c_local + 1) * 128],
                            rhs=w_sb[:],
                            start=True,
                            stop=True,
                        )
                        dst = out_sb[:, j * KW:(j + 1) * KW]
                        if copy_engines[(g * G + j) % 2] == "v":
                            nc.vector.tensor_copy(dst, ps[:])
                        else:
                            nc.scalar.activation(
                                dst, ps[:], mybir.ActivationFunctionType.Copy
                            )
                    dma_engines[g % len(dma_engines)].dma_start(y[g], out_sb[:])
    nc.compile()
    return nc


def _get_nc():
    global _NC
    if _NC is None:
        _NC = _build_nc()
    return _NC


def kernel(input, coords, W, bias):
    global LAST_RESULTS
    feats = np.ascontiguousarray(input, dtype=np.float32)
    # W_all[c, k*16+o] = W[k, c, o]
    w_all = np.ascontiguousarray(np.transpose(W, (1, 0, 2)).reshape(C_IN, KW))

    in_maps = []
    for i in range(N_CORES):
        sh = feats[i * PER:(i + 1) * PER]  # [25000, 64]
        fT = np.zeros((C_IN, PAD), np.float32)
        fT[:, :PER] = sh.T
        in_maps.append({"featsT": fT, "w_all": w_all})

    nc = _get_nc()
    res = run_bass_kernel_spmd(nc, in_maps, list(range(N_CORES)), trace=TRACE)
    LAST_RESULTS = res

    # Reassemble per-core Y: [28, 128, 7*432] -> [25088, 432] -> [25000, 432]
    parts = []
    for i in range(N_CORES):
        yc = np.asarray(res.results[i]["y"]).reshape(NGRP, 128, G, KW)
        yc = yc.transpose(0, 2, 1, 3).reshape(PAD, KW)[:PER]
        parts.append(yc)
    Y = np.concatenate(parts, axis=0)          # [N, 432]
    C = Y.reshape(NK, C_OUT)                   # row n*27+k

    # Host rulebook: output coords per (point, tap)
    xyz = coords[:, 1:4].astype(np.int32)
    r = np.arange(3, dtype=np.int32)
    off = np.stack(np.meshgrid(r, r, r, indexing="ij"), axis=-1).reshape(K3, 3)
    oc = xyz[:, None, :] - off[None, :, :]     # [N, 27, 3]
    sp = np.array(OUT_SP, dtype=np.int32)
    valid = np.all((oc >= 0) & (oc < sp), axis=-1)  # [N, 27]
    lin = (
        oc[..., 0] * (OUT_SP[1] * OUT_SP[2])
        + oc[..., 1] * OUT_SP[2]
        + oc[..., 2]
    )
    lin = np.where(valid, lin, SENT).reshape(-1)    # [NK] int32

    order = np.argsort(lin, kind="stable")
    slin = lin[order]
    starts = np.flatnonzero(np.r_[True, slin[1:] != slin[:-1]])
    uniq_vals = slin[starts]
    U = len(starts)

    Csort = C[order]
    sums = np.add.reduceat(Csort, starts, axis=0)   # [U, 16]

    out = np.zeros((NK, C_OUT), np.float32)
    out[:U] = sums + bias[None, :].astype(np.float32)
    if uniq_vals[-1] == SENT:
        out[U - 1] = 0.0

    uniq = np.full(NK, SENT, np.int32)
    uniq[:U] = uniq_vals
    return out, uniq


# revision 12
# speedup vs baseline: 1.9928x; 1.9928x over previous
import sys

sys.path.insert(0, "/opt/trn_rl_repo")

import ml_dtypes
import numpy as np

from concourse import bass, bacc, mybir
import concourse.tile as tile
from concourse.bass_utils import run_bass_kernel_spmd

# Problem constants (hardcoded; kernel.py must be self-contained).
N = 200000
C_IN = 64
C_OUT = 16
K3 = 27
KW = K3 * C_OUT  # 432
OUT_SP = (998, 998, 38)
SENT = OUT_SP[0] * OUT_SP[1] * OUT_SP[2]  # 37848152
NK = N * K3

N_CORES = 8
PER = N // N_CORES            # 25000 points per core
PAD = 25088                   # 196 * 128
NCHUNK = PAD // 128           # 196
G = 7                         # chunks per output group
NGRP = NCHUNK // G            # 28
N_IN_TILES = 4
TILE_F = PAD // N_IN_TILES    # 6272 points per input tile
CHUNKS_PER_TILE = TILE_F // 128  # 49
GRPS_PER_TILE = CHUNKS_PER_TILE // G  # 7

TRACE = False
LAST_RESULTS = None

_NC = None


def _build_nc():
    nc = bacc.Bacc("TRN2", target_bir_lowering=False, debug=False)
    featsT = nc.dram_tensor("featsT", [C_IN, PAD], mybir.dt.bfloat16, kind="ExternalInput")
    w_all = nc.dram_tensor("w_all", [C_IN, KW], mybir.dt.bfloat16, kind="ExternalInput")
    # y[g, p, j*432+o] = contribution row n = (g*7+j)*128 + p
    y = nc.dram_tensor("y", [NGRP, 128, G * KW], mybir.dt.float32, kind="ExternalOutput")

    with tile.TileContext(nc) as tc:
        with (
            tc.tile_pool(name="const", bufs=1) as constp,
            tc.tile_pool(name="inp", bufs=2) as inp,
            tc.tile_pool(name="outp", bufs=3) as outp,
            tc.tile_pool(name="psum", bufs=8, space="PSUM") as psump,
        ):
            w_sb = constp.tile([C_IN, KW], mybir.dt.bfloat16)
            nc.sync.dma_start(w_sb[:], w_all[:])

            copy_engines = ["v", "s"]
            dma_engines = [nc.sync, nc.scalar]

            for t in range(N_IN_TILES):
                ftile = inp.tile([C_IN, TILE_F], mybir.dt.bfloat16)
                nc.sync.dma_start(ftile[:], featsT[:, t * TILE_F:(t + 1) * TILE_F])
                for gg in range(GRPS_PER_TILE):
                    g = t * GRPS_PER_TILE + gg
                    out_sb = outp.tile([128, G * KW], mybir.dt.float32)
                    for j in range(G):
                        c_local = gg * G + j
                        ps = psump.tile([128, KW], mybir.dt.float32)
                        nc.tensor.matmul(
                            ps[:],
                            lhsT=ftile[:, c_local * 128:(c_local + 1) * 128],
                            rhs=w_sb[:],
                            start=True,
                            stop=True,
                        )
                        dst = out_sb[:, j * KW:(j + 1) * KW]
                        if copy_engines[(g * G + j) % 2] == "v":
                            nc.vector.tensor_copy(dst, ps[:])
                        else:
                            nc.scalar.activation(
                                dst, ps[:], mybir.ActivationFunctionType.Copy
                            )
                    dma_engines[g % len(dma_engines)].dma_start(y[g], out_sb[:])
    nc.compile()
    return nc


def _get_nc():
    global _NC
    if _NC is None:
        _NC = _build_nc()
    return _NC


def kernel(input, coords, W, bias):
    global LAST_RESULTS
    bf16 = ml_dtypes.bfloat16
    feats = np.asarray(input, dtype=np.float32).astype(bf16)
    # W_all[c, k*16+o] = W[k, c, o]
    w_all = np.ascontiguousarray(
        np.transpose(np.asarray(W, np.float32), (1, 0, 2)).reshape(C_IN, KW)
    ).astype(bf16)

    in_maps = []
    for i in range(N_CORES):
        sh = feats[i * PER:(i + 1) * PER]  # [25000, 64]
        fT = np.zeros((C_IN, PAD), bf16)
        fT[:, :PER] = sh.T
        in_maps.append({"featsT": fT, "w_all": w_all})

    nc = _get_nc()
    res = run_bass_kernel_spmd(nc, in_maps, list(range(N_CORES)), trace=TRACE)
    LAST_RESULTS = res

    # Reassemble per-core Y: [28, 128, 7*432] -> [25088, 432] -> [25000, 432]
    parts = []
    for i in range(N_CORES):
        yc = np.asarray(res.results[i]["y"]).reshape(NGRP, 128, G, KW)
        yc = yc.transpose(0, 2, 1, 3).reshape(PAD, KW)[:PER]
        parts.append(yc)
    Y = np.concatenate(parts, axis=0)          # [N, 432]
    C = Y.reshape(NK, C_OUT)                   # row n*27+k

    # Host rulebook: output coords per (point, tap)
    xyz = coords[:, 1:4].astype(np.int32)
    r = np.arange(3, dtype=np.int32)
    off = np.stack(np.meshgrid(r, r, r, indexing="ij"), axis=-1).reshape(K3, 3)
    oc = xyz[:, None, :] - off[None, :, :]     # [N, 27, 3]
    sp = np.array(OUT_SP, dtype=np.int32)
    valid = np.all((oc >= 0) & (oc < sp), axis=-1)  # [N, 27]
    lin = (
        oc[..., 0] * (OUT_SP[1] * OUT_SP[2])
        + oc[..., 1] * OUT_SP[2]
        + oc[..., 2]
    )
    lin = np.where(valid, lin, SENT).reshape(-1)    # [NK] int32

    order = np.argsort(lin, kind="stable")
    slin = lin[order]
    starts = np.flatnonzero(np.r_[True, slin[1:] != slin[:-1]])
    uniq_vals = slin[starts]
    U = len(starts)

    Csort = C[order]
    sums = np.add.reduceat(Csort, starts, axis=0)   # [U, 16]

    out = np.zeros((NK, C_OUT), np.float32)
    out[:U] = sums + bias[None, :].astype(np.float32)
    if uniq_vals[-1] == SENT:
        out[U - 1] = 0.0

    uniq = np.full(NK, SENT, np.int32)
    uniq[:U] = uniq_vals
    return out, uniq


# revision 19
# speedup vs baseline: 3.1171x; 1.5642x over previous
import sys

sys.path.insert(0, "/opt/trn_rl_repo")

import numpy as np

from concourse import bass, bacc, mybir
import concourse.tile as tile
from concourse.bass_utils import run_bass_kernel_spmd

# Problem constants (hardcoded; kernel.py must be self-contained).
N = 200000
C_IN = 64
C_OUT = 16
K3 = 27
KW = K3 * C_OUT  # 432
OUT_SP = (998, 998, 38)
SENT = OUT_SP[0] * OUT_SP[1] * OUT_SP[2]  # 37848152
NK = N * K3

N_CORES = 8
PER = N // N_CORES            # 25000 points per core
PAD = 25088                   # 196 * 128
NCHUNK = PAD // 128           # 196
G = 7                         # chunks per output group
NGRP = NCHUNK // G            # 28
N_IN_TILES = 4
TILE_F = PAD // N_IN_TILES    # 6272 points per input tile
CHUNKS_PER_TILE = TILE_F // 128  # 49
GRPS_PER_TILE = CHUNKS_PER_TILE // G  # 7

TRACE = False
LAST_RESULTS = None

_NC = None


def _build_nc():
    nc = bacc.Bacc("TRN2", target_bir_lowering=False, debug=False)
    featsT = nc.dram_tensor("featsT", [C_IN, PAD], mybir.dt.float16, kind="ExternalInput")
    w_all = nc.dram_tensor("w_all", [C_IN, KW], mybir.dt.float16, kind="ExternalInput")
    # y[g, p, j*432+o] = contribution row n = (g*7+j)*128 + p
    y = nc.dram_tensor("y", [NGRP, 128, G * KW], mybir.dt.float16, kind="ExternalOutput")

    with tile.TileContext(nc) as tc:
        with (
            tc.tile_pool(name="const", bufs=1) as constp,
            tc.tile_pool(name="inp", bufs=2) as inp,
            tc.tile_pool(name="outp", bufs=3) as outp,
            tc.tile_pool(name="psum", bufs=8, space="PSUM") as psump,
        ):
            w_sb = constp.tile([C_IN, KW], mybir.dt.float16)
            nc.sync.dma_start(w_sb[:], w_all[:])

            copy_engines = ["v", "s"]
            dma_engines = [nc.sync, nc.scalar]

            for t in range(N_IN_TILES):
                ftile = inp.tile([C_IN, TILE_F], mybir.dt.float16)
                nc.sync.dma_start(ftile[:], featsT[:, t * TILE_F:(t + 1) * TILE_F])
                for gg in range(GRPS_PER_TILE):
                    g = t * GRPS_PER_TILE + gg
                    out_sb = outp.tile([128, G * KW], mybir.dt.float16)
                    for j in range(G):
                        c_local = gg * G + j
                        ps = psump.tile([128, KW], mybir.dt.float32)
                        nc.tensor.matmul(
                            ps[:],
                            lhsT=ftile[:, c_local * 128:(c_local + 1) * 128],
                            rhs=w_sb[:],
                            start=True,
                            stop=True,
                        )
                        dst = out_sb[:, j * KW:(j + 1) * KW]
                        if copy_engines[(g * G + j) % 2] == "v":
                            nc.vector.tensor_copy(dst, ps[:])
                        else:
                            nc.scalar.activation(
                                dst, ps[:], mybir.ActivationFunctionType.Copy
                            )
                    dma_engines[g % len(dma_engines)].dma_start(y[g], out_sb[:])
    nc.compile()
    return nc


def _get_nc():
    global _NC
    if _NC is None:
        _NC = _build_nc()
    return _NC


def kernel(input, coords, W, bias):
    global LAST_RESULTS
    feats = np.asarray(input, dtype=np.float32).astype(np.float16)
    # W_all[c, k*16+o] = W[k, c, o]
    w_all = np.ascontiguousarray(
        np.transpose(np.asarray(W, np.float32), (1, 0, 2)).reshape(C_IN, KW)
    ).astype(np.float16)

    in_maps = []
    for i in range(N_CORES):
        sh = feats[i * PER:(i + 1) * PER]  # [25000, 64]
        fT = np.zeros((C_IN, PAD), np.float16)
        fT[:, :PER] = sh.T
        in_maps.append({"featsT": fT, "w_all": w_all})

    nc = _get_nc()
    res = run_bass_kernel_spmd(nc, in_maps, list(range(N_CORES)), trace=TRACE)
    LAST_RESULTS = res

    # Reassemble per-core Y: [28, 128, 7*432] -> [25088, 432] -> [25000, 432]
    parts = []
    for i in range(N_CORES):
        yc = np.asarray(res.results[i]["y"]).astype(np.float32).reshape(NGRP, 128, G, KW)
        yc = yc.transpose(0, 2, 1, 3).reshape(PAD, KW)[:PER]
        parts.append(yc)
    Y = np.concatenate(parts, axis=0)          # [N, 432]
    C = Y.reshape(NK, C_OUT)                   # row n*27+k

    # Host rulebook: output coords per (point, tap)
    xyz = coords[:, 1:4].astype(np.int32)
    r = np.arange(3, dtype=np.int32)
    off = np.stack(np.meshgrid(r, r, r, indexing="ij"), axis=-1).reshape(K3, 3)
    oc = xyz[:, None, :] - off[None, :, :]     # [N, 27, 3]
    sp = np.array(OUT_SP, dtype=np.int32)
    valid = np.all((oc >= 0) & (oc < sp), axis=-1)  # [N, 27]
    lin = (
        oc[..., 0] * (OUT_SP[1] * OUT_SP[2])
        + oc[..., 1] * OUT_SP[2]
        + oc[..., 2]
    )
    lin = np.where(valid, lin, SENT).reshape(-1)    # [NK] int32

    order = np.argsort(lin, kind="stable")
    slin = lin[order]
    starts = np.flatnonzero(np.r_[True, slin[1:] != slin[:-1]])
    uniq_vals = slin[starts]
    U = len(starts)

    Csort = C[order]
    sums = np.add.reduceat(Csort, starts, axis=0)   # [U, 16]

    out = np.zeros((NK, C_OUT), np.float32)
    out[:U] = sums + bias[None, :].astype(np.float32)
    if uniq_vals[-1] == SENT:
        out[U - 1] = 0.0

    uniq = np.full(NK, SENT, np.int32)
    uniq[:U] = uniq_vals
    return out, uniq


# revision 24
# speedup vs baseline: 3.2781x; 1.0516x over previous
import sys

sys.path.insert(0, "/opt/trn_rl_repo")

import numpy as np

from concourse import bass, bacc, mybir
import concourse.tile as tile
from concourse.bass_utils import run_bass_kernel_spmd

# Problem constants (hardcoded; kernel.py must be self-contained).
N = 200000
C_IN = 64
C_OUT = 16
K3 = 27
KW = K3 * C_OUT  # 432
OUT_SP = (998, 998, 38)
SENT = OUT_SP[0] * OUT_SP[1] * OUT_SP[2]  # 37848152
NK = N * K3

N_CORES = 8
PER = N // N_CORES            # 25000 points per core
PAD = 25088                   # 196 * 128
NCHUNK = PAD // 128           # 196
G = 7                         # chunks per output group
NGRP = NCHUNK // G            # 28
N_IN_TILES = 4
TILE_F = PAD // N_IN_TILES    # 6272 points per input tile
CHUNKS_PER_TILE = TILE_F // 128  # 49
GRPS_PER_TILE = CHUNKS_PER_TILE // G  # 7

TRACE = False
LAST_RESULTS = None

_NC = None


def _build_nc():
    nc = bacc.Bacc("TRN2", target_bir_lowering=False, debug=False)
    featsT = nc.dram_tensor("featsT", [C_IN, PAD], mybir.dt.float16, kind="ExternalInput")
    w_all = nc.dram_tensor("w_all", [C_IN, KW], mybir.dt.float16, kind="ExternalInput")
    # y[g, p, j*432+o] = contribution row n = (g*7+j)*128 + p
    y = nc.dram_tensor("y", [NGRP, 128, G * KW], mybir.dt.float16, kind="ExternalOutput")

    with tile.TileContext(nc) as tc:
        with (
            tc.tile_pool(name="const", bufs=1) as constp,
            tc.tile_pool(name="inp", bufs=1) as inp,
            tc.tile_pool(name="outp", bufs=3) as outp,
            tc.tile_pool(name="psum", bufs=8, space="PSUM") as psump,
        ):
            w_sb = constp.tile([C_IN, KW], mybir.dt.float16)
            nc.sync.dma_start(w_sb[:], w_all[:])

            copy_engines = ["v", "s"]

            ftiles = [
                inp.tile([C_IN, TILE_F], mybir.dt.float16, name=f"ftile{t}")
                for t in range(N_IN_TILES)
            ]
            for t in range(N_IN_TILES):
                nc.sync.dma_start(
                    ftiles[t][:], featsT[:, t * TILE_F:(t + 1) * TILE_F]
                )

            for t in range(N_IN_TILES):
                ftile = ftiles[t]
                for gg in range(GRPS_PER_TILE):
                    g = t * GRPS_PER_TILE + gg
                    out_sb = outp.tile([128, G * KW], mybir.dt.float16)
                    for j in range(G):
                        c_local = gg * G + j
                        ps = psump.tile([128, KW], mybir.dt.float32)
                        nc.tensor.matmul(
                            ps[:],
                            lhsT=ftile[:, c_local * 128:(c_local + 1) * 128],
                            rhs=w_sb[:],
                            start=True,
                            stop=True,
                        )
                        dst = out_sb[:, j * KW:(j + 1) * KW]
                        if copy_engines[(g * G + j) % 2] == "v":
                            nc.vector.tensor_copy(dst, ps[:])
                        else:
                            nc.scalar.activation(
                                dst, ps[:], mybir.ActivationFunctionType.Copy
                            )
                    nc.sync.dma_start(y[g], out_sb[:])
    nc.compile()
    return nc


def _get_nc():
    global _NC
    if _NC is None:
        _NC = _build_nc()
    return _NC


def kernel(input, coords, W, bias):
    global LAST_RESULTS
    feats = np.asarray(input, dtype=np.float32).astype(np.float16)
    # W_all[c, k*16+o] = W[k, c, o]
    w_all = np.ascontiguousarray(
        np.transpose(np.asarray(W, np.float32), (1, 0, 2)).reshape(C_IN, KW)
    ).astype(np.float16)

    in_maps = []
    for i in range(N_CORES):
        sh = feats[i * PER:(i + 1) * PER]  # [25000, 64]
        fT = np.zeros((C_IN, PAD), np.float16)
        fT[:, :PER] = sh.T
        in_maps.append({"featsT": fT, "w_all": w_all})

    nc = _get_nc()
    res = run_bass_kernel_spmd(nc, in_maps, list(range(N_CORES)), trace=TRACE)
    LAST_RESULTS = res

    # Reassemble per-core Y: [28, 128, 7*432] -> [25088, 432] -> [25000, 432]
    parts = []
    for i in range(N_CORES):
        yc = np.asarray(res.results[i]["y"]).astype(np.float32).reshape(NGRP, 128, G, KW)
        yc = yc.transpose(0, 2, 1, 3).reshape(PAD, KW)[:PER]
        parts.append(yc)
    Y = np.concatenate(parts, axis=0)          # [N, 432]
    C = Y.reshape(NK, C_OUT)                   # row n*27+k

    # Host rulebook: output coords per (point, tap)
    xyz = coords[:, 1:4].astype(np.int32)
    r = np.arange(3, dtype=np.int32)
    off = np.stack(np.meshgrid(r, r, r, indexing="ij"), axis=-1).reshape(K3, 3)
    oc = xyz[:, None, :] - off[None, :, :]     # [N, 27, 3]
    sp = np.array(OUT_SP, dtype=np.int32)
    valid = np.all((oc >= 0) & (oc < sp), axis=-1)  # [N, 27]
    lin = (
        oc[..., 0] * (OUT_SP[1] * OUT_SP[2])
        + oc[..., 1] * OUT_SP[2]
        + oc[..., 2]
    )
    lin = np.where(valid, lin, SENT).reshape(-1)    # [NK] int32

    order = np.argsort(lin, kind="stable")
    slin = lin[order]
    starts = np.flatnonzero(np.r_[True, slin[1:] != slin[:-1]])
    uniq_vals = slin[starts]
    U = len(starts)

    Csort = C[order]
    sums = np.add.reduceat(Csort, starts, axis=0)   # [U, 16]

    out = np.zeros((NK, C_OUT), np.float32)
    out[:U] = sums + bias[None, :].astype(np.float32)
    if uniq_vals[-1] == SENT:
        out[U - 1] = 0.0

    uniq = np.full(NK, SENT, np.int32)
    uniq[:U] = uniq_vals
    return out, uniq


# revision 26
# speedup vs baseline: 3.5091x; 1.0705x over previous
import sys

sys.path.insert(0, "/opt/trn_rl_repo")

import numpy as np

from concourse import bass, bacc, mybir
import concourse.tile as tile
from concourse.bass_utils import run_bass_kernel_spmd

# Problem constants (hardcoded; kernel.py must be self-contained).
N = 200000
C_IN = 64
C_OUT = 16
K3 = 27
KW = K3 * C_OUT  # 432
OUT_SP = (998, 998, 38)
SENT = OUT_SP[0] * OUT_SP[1] * OUT_SP[2]  # 37848152
NK = N * K3

N_CORES = 8
PER = N // N_CORES            # 25000 points per core
PAD = 25088                   # 196 * 128
NCHUNK = PAD // 128           # 196
G = 7                         # chunks per output group
NGRP = NCHUNK // G            # 28
N_IN_TILES = 14               # 2 groups (1792 points) per input tile
TILE_F = PAD // N_IN_TILES    # 1792
GRPS_PER_TILE = NGRP // N_IN_TILES  # 2
PREFETCH = 3                  # input tiles issued before the compute loop

TRACE = False
LAST_RESULTS = None

_NC = None


def _build_nc():
    nc = bacc.Bacc("TRN2", target_bir_lowering=False, debug=False)
    featsT = nc.dram_tensor("featsT", [C_IN, PAD], mybir.dt.float16, kind="ExternalInput")
    w_all = nc.dram_tensor("w_all", [C_IN, KW], mybir.dt.float16, kind="ExternalInput")
    # y[g, p, j*432+o] = contribution row n = (g*7+j)*128 + p
    y = nc.dram_tensor("y", [NGRP, 128, G * KW], mybir.dt.float16, kind="ExternalOutput")

    with tile.TileContext(nc) as tc:
        with (
            tc.tile_pool(name="const", bufs=1) as constp,
            tc.tile_pool(name="inp", bufs=1) as inp,
            tc.tile_pool(name="outp", bufs=3) as outp,
            tc.tile_pool(name="psum", bufs=8, space="PSUM") as psump,
        ):
            w_sb = constp.tile([C_IN, KW], mybir.dt.float16)
            nc.sync.dma_start(w_sb[:], w_all[:])

            copy_engines = ["v", "s"]

            ftiles = [
                inp.tile([C_IN, TILE_F], mybir.dt.float16, name=f"ftile{t}")
                for t in range(N_IN_TILES)
            ]

            def fetch(t):
                nc.sync.dma_start(
                    ftiles[t][:], featsT[:, t * TILE_F:(t + 1) * TILE_F]
                )

            for t in range(PREFETCH):
                fetch(t)

            for g in range(NGRP):
                t = g // GRPS_PER_TILE
                gg = g % GRPS_PER_TILE
                if gg == 0 and t + PREFETCH < N_IN_TILES:
                    fetch(t + PREFETCH)
                ftile = ftiles[t]
                out_sb = outp.tile([128, G * KW], mybir.dt.float16)
                for j in range(G):
                    c_local = gg * G + j
                    ps = psump.tile([128, KW], mybir.dt.float32)
                    nc.tensor.matmul(
                        ps[:],
                        lhsT=ftile[:, c_local * 128:(c_local + 1) * 128],
                        rhs=w_sb[:],
                        start=True,
                        stop=True,
                    )
                    dst = out_sb[:, j * KW:(j + 1) * KW]
                    if copy_engines[(g * G + j) % 2] == "v":
                        nc.vector.tensor_copy(dst, ps[:])
                    else:
                        nc.scalar.activation(
                            dst, ps[:], mybir.ActivationFunctionType.Copy
                        )
                nc.sync.dma_start(y[g], out_sb[:])
    nc.compile()
    return nc


def _get_nc():
    global _NC
    if _NC is None:
        _NC = _build_nc()
    return _NC


def kernel(input, coords, W, bias):
    global LAST_RESULTS
    feats = np.asarray(input, dtype=np.float32).astype(np.float16)
    # W_all[c, k*16+o] = W[k, c, o]
    w_all = np.ascontiguousarray(
        np.transpose(np.asarray(W, np.float32), (1, 0, 2)).reshape(C_IN, KW)
    ).astype(np.float16)

    in_maps = []
    for i in range(N_CORES):
        sh = feats[i * PER:(i + 1) * PER]  # [25000, 64]
        fT = np.zeros((C_IN, PAD), np.float16)
        fT[:, :PER] = sh.T
        in_maps.append({"featsT": fT, "w_all": w_all})

    nc = _get_nc()
    res = run_bass_kernel_spmd(nc, in_maps, list(range(N_CORES)), trace=TRACE)
    LAST_RESULTS = res

    # Reassemble per-core Y: [28, 128, 7*432] -> [25088, 432] -> [25000, 432]
    parts = []
    for i in range(N_CORES):
        yc = np.asarray(res.results[i]["y"]).astype(np.float32).reshape(NGRP, 128, G, KW)
        yc = yc.transpose(0, 2, 1, 3).reshape(PAD, KW)[:PER]
        parts.append(yc)
    Y = np.concatenate(parts, axis=0)          # [N, 432]
    C = Y.reshape(NK, C_OUT)                   # row n*27+k

    # Host rulebook: output coords per (point, tap)
    xyz = coords[:, 1:4].astype(np.int32)
    r = np.arange(3, dtype=np.int32)
    off = np.stack(np.meshgrid(r, r, r, indexing="ij"), axis=-1).reshape(K3, 3)
    oc = xyz[:, None, :] - off[None, :, :]     # [N, 27, 3]
    sp = np.array(OUT_SP, dtype=np.int32)
    valid = np.all((oc >= 0) & (oc < sp), axis=-1)  # [N, 27]
    lin = (
        oc[..., 0] * (OUT_SP[1] * OUT_SP[2])
        + oc[..., 1] * OUT_SP[2]
        + oc[..., 2]
    )
    lin = np.where(valid, lin, SENT).reshape(-1)    # [NK] int32

    order = np.argsort(lin, kind="stable")
    slin = lin[order]
    starts = np.flatnonzero(np.r_[True, slin[1:] != slin[:-1]])
    uniq_vals = slin[starts]
    U = len(starts)

    Csort = C[order]
    sums = np.add.reduceat(Csort, starts, axis=0)   # [U, 16]

    out = np.zeros((NK, C_OUT), np.float32)
    out[:U] = sums + bias[None, :].astype(np.float32)
    if uniq_vals[-1] == SENT:
        out[U - 1] = 0.0

    uniq = np.full(NK, SENT, np.int32)
    uniq[:U] = uniq_vals
    return out, uniq
